# revision 18
# baseline (speedup 1.0000x reference)
"""Deformable PS-ROI pooling on Trainium2 (Bass/Tile), SPMD over 8 cores.

v3 strategy (host-precomputed dedup gather, pixel-pair records):
  The 784 bilinear corner reads per roi (49 bins x 2x2 samples x 4 corners)
  hit only ~105 unique feature-map pixels on average, and those cluster into
  ~55 aligned pixel PAIRS (rows are contiguous in the channel-last layout).
  The host dedups terms into per-roi unique aligned pairs and a folded
  weight matrix (bilinear x validity x 1/count), pairs rois (large+small)
  into 2-roi groups, and deals groups across cores so all 8 cores run one
  identical SPMD program.  The device executes: int16-indexed dma_gather of
  bf16 pair records (1 KiB/descriptor, ~35 chunks of 128 pairs) -> per group
  PE matmuls mask[128,98]^T x pixels[128,256] (two per chunk: even/odd pair
  halves) accumulated in PSUM -> DVE copy to bf16 -> batched output DMA
  (one per 4 groups, partition-major out layout).
"""

import math

import ml_dtypes
import numpy as np

import concourse.bacc as bacc
import concourse.mybir as mybir
from concourse import tile
from concourse.bass_utils import run_bass_kernel_spmd

F32 = mybir.dt.float32
BF16 = mybir.dt.bfloat16
I16 = mybir.dt.int16

N_CORES = 8
N_ROIS = 512
P = 7                   # pooled output size
NB = P * P              # 49 bins
CH = 256                # channels
H = W = 128             # feature map spatial
B = 2                   # batch
NPX = B * H * W         # 32768 flat pixels; 16384 aligned pairs
S = 2                   # samples per part (per axis)
RPC = N_ROIS // N_CORES         # 64 rois per core
GPC = RPC // 2                  # 32 groups (of 2 rois) per core
OBATCH = 4                      # groups per output DMA
CPG_MAX = 8                     # chunks per gather instruction (1024 descs)
SCALE = 0.0625
TRANS_STD = 0.1


def _gather_splits(total_chunks):
    """Split the chunk stream into gather instructions (<=8 chunks each),
    sized so the round-robin queue assignment balances chunk load."""
    nsplit = math.ceil(total_chunks / CPG_MAX)
    nsplit = math.ceil(nsplit / 4) * 4  # multiple of 4 for queue balance
    base = total_chunks // nsplit
    rem = total_chunks - base * nsplit
    return [base + (1 if k < rem else 0) for k in range(nsplit)]


def build_program(schedule, reps: int = 1, mode: str = "full"):
    """schedule: tuple of chunks-per-group (len GPC), identical on all cores.
    One chunk = 128 aligned pixel pairs.  mode: 'full' | 'gather' | 'compute'
    ('gather'/'compute' are timing-only variants used by the bench)."""
    total_chunks = sum(schedule)
    splits = _gather_splits(total_chunks)
    ngather = len(splits)

    nc = bacc.Bacc("TRN2", target_bir_lowering=False, debug=False, num_swdge_queues=4)
    nc.dynamic_dma_scratch_size = 2 ** 16

    data = nc.dram_tensor("data_t", [NPX // 2, 2 * CH], BF16, kind="ExternalInput")
    idx_d = nc.dram_tensor("idxg", [128, total_chunks * 8], I16,
                           kind="ExternalInput")
    msk_d = nc.dram_tensor("masks", [128, total_chunks * 4 * NB], BF16,
                           kind="ExternalInput")
    # partition-major output: out[p, g*CH + c] for bin-row p of group g
    out_d = nc.dram_tensor("out", [2 * NB, GPC * CH], BF16, kind="ExternalOutput")

    with tile.TileContext(nc) as tc:
        with (
            tc.tile_pool(name="const", bufs=1) as cst,
            tc.tile_pool(name="gp", bufs=ngather + 2) as gp,
            tc.tile_pool(name="obp", bufs=3) as obp,
            tc.tile_pool(name="psp", bufs=6, space="PSUM") as psp,
        ):
            idxg = cst.tile([128, total_chunks * 8], I16)
            nc.sync.dma_start(idxg[:, :], idx_d.ap())
            msk = cst.tile([128, total_chunks * 4 * NB], BF16)
            nc.sync.dma_start(msk[:, :], msk_d.ap())

            from contextlib import nullcontext
            if mode == "compute":
                gt0 = cst.tile([128, max(splits) * 2 * CH], BF16)
                nc.vector.memset(gt0[:, :], 0.0)

            split_base = [0]
            for s_ in splits:
                split_base.append(split_base[-1] + s_)

            loop_cm = tc.For_i(0, reps, 1) if reps > 1 else nullcontext()
            with loop_cm:
                gts = []
                if mode in ("full", "gather"):
                    for k in range(ngather):
                        nch = splits[k]
                        gt = gp.tile([128, nch * 2 * CH], BF16)
                        dest = gt[:, :].rearrange("p (j f) -> p j f", f=2 * CH)
                        nc.gpsimd.dma_gather(
                            dest,
                            data.ap(),
                            idxg[:, split_base[k] * 8 : split_base[k + 1] * 8],
                            nch * 128,
                            nch * 128,
                            2 * CH,
                            queue_num=k % 4,
                            single_packet=False,
                        )
                        gts.append(gt)
                else:
                    gts = [gt0] * ngather

                def chunk_view(c):
                    k = 0
                    while split_base[k + 1] <= c:
                        k += 1
                    return gts[k], c - split_base[k]

                chunk = 0
                for t in range(GPC // OBATCH) if mode != "gather" else []:
                    sb = obp.tile([2 * NB, OBATCH * CH], BF16)
                    for i in range(OBATCH):
                        g = t * OBATCH + i
                        ncg = schedule[g]
                        ps = psp.tile([2 * NB, CH], F32)
                        nmm = 2 * ncg
                        for j in range(ncg):
                            c = chunk + j
                            gt, sl = chunk_view(c)
                            for s in range(2):
                                gv = gt[:, sl * 2 * CH + s * CH
                                        : sl * 2 * CH + (s + 1) * CH]
                                mm = 2 * j + s
                                nc.tensor.matmul(
                                    ps[:, :],
                                    msk[:, (c * 2 + s) * 2 * NB
                                        : (c * 2 + s + 1) * 2 * NB],
                                    gv,
                                    start=(mm == 0),
                                    stop=(mm == nmm - 1),
                                )
                        chunk += ncg
                        nc.vector.tensor_copy(
                            sb[:, i * CH : (i + 1) * CH], ps[:, :]
                        )
                    nc.sync.dma_start(
                        out_d.ap()[:, t * OBATCH * CH : (t + 1) * OBATCH * CH],
                        sb[:, :],
                    )

    nc.finalize()
    return nc


def _precompute(rois, offset):
    """Host-side: per-roi unique aligned pixel pairs + folded weight masks,
    paired and dealt into a uniform per-core schedule."""
    N = rois.shape[0]
    bidx = rois[:, 0].astype(np.int64)
    sw = np.round(rois[:, 1]) * SCALE - 0.5
    sh = np.round(rois[:, 2]) * SCALE - 0.5
    ew = (np.round(rois[:, 3]) + 1.0) * SCALE - 0.5
    eh = (np.round(rois[:, 4]) + 1.0) * SCALE - 0.5
    rw = np.maximum(ew - sw, 0.1)
    rh = np.maximum(eh - sh, 0.1)
    bw, bh = rw / P, rh / P
    subw, subh = bw / S, bh / S
    pw = np.arange(P)
    ph = np.arange(P)
    tx = offset[:, 0] * TRANS_STD
    ty = offset[:, 1] * TRANS_STD
    ws = pw[None, None, :] * bw[:, None, None] + sw[:, None, None] \
        + tx * rw[:, None, None]
    hs = ph[None, :, None] * bh[:, None, None] + sh[:, None, None] \
        + ty * rh[:, None, None]
    w = ws[..., None, None] + np.arange(S)[None, None, None, None, :] \
        * subw[:, None, None, None, None]
    h = hs[..., None, None] + np.arange(S)[None, None, None, :, None] \
        * subh[:, None, None, None, None]
    valid = (w >= -0.5) & (w <= W - 0.5) & (h >= -0.5) & (h <= H - 0.5)
    wc = np.clip(w, 0, W - 1)
    hc = np.clip(h, 0, H - 1)
    x1 = np.floor(wc).astype(np.int64)
    y1 = np.floor(hc).astype(np.int64)
    x2 = np.minimum(x1 + 1, W - 1)
    y2 = np.minimum(y1 + 1, H - 1)
    dx = (wc - x1).astype(np.float32)
    dy = (hc - y1).astype(np.float32)
    count = valid.sum(axis=(-1, -2)).astype(np.float32)
    inv = np.where(count > 0, 1.0 / np.maximum(count, 1.0), 0.0)
    v = valid.astype(np.float32) * inv[:, :, :, None, None]
    w11 = (1 - dx) * (1 - dy) * v
    w12 = (1 - dx) * dy * v
    w21 = dx * (1 - dy) * v
    w22 = dx * dy * v
    base = (bidx * (H * W))[:, None, None, None, None]

    def flat(y, x):
        return base + y * W + x

    pix = np.stack([flat(y1, x1), flat(y2, x1), flat(y1, x2), flat(y2, x2)],
                   axis=-1).reshape(N, NB, 16)
    wt = np.stack([w11, w12, w21, w22], axis=-1).reshape(N, NB, 16)

    # dedup at aligned-pair granularity; mask rows keyed (pair, sub)
    upair, umask, ucnt = [], [], np.zeros(N, np.int64)
    binc = np.repeat(np.arange(NB), 16)
    for n in range(N):
        pairs_n = pix[n] // 2
        subs_n = pix[n] & 1
        u, inv_idx = np.unique(pairs_n, return_inverse=True)
        m = np.zeros((len(u), 2, NB), np.float32)
        np.add.at(m, (inv_idx.reshape(-1), subs_n.reshape(-1), binc),
                  wt[n].reshape(-1))
        upair.append(u)
        umask.append(m)
        ucnt[n] = len(u)

    # pair rois -> groups, capacity-aware: largest partner that keeps the
    # pair-count sum within one 128-pair chunk; else largest fitting two
    # chunks (minimizes padded chunk slots)
    avail = [int(x) for x in np.argsort(-ucnt, kind="stable")]
    pairs = []
    while avail:
        a = avail.pop(0)
        b = None
        for cap in (128, 256, 1 << 30):
            for i, x in enumerate(avail):
                if ucnt[a] + ucnt[x] <= cap:
                    b = avail.pop(i)
                    break
            if b is not None:
                break
        pairs.append((a, b))
    gcost = np.array([int(math.ceil((ucnt[a] + ucnt[b]) / 128)) for a, b in pairs])
    grank = np.argsort(-gcost, kind="stable")
    schedule = tuple(
        int(gcost[grank[s * N_CORES : (s + 1) * N_CORES]].max())
        for s in range(GPC)
    )
    total_chunks = sum(schedule)
    splits = _gather_splits(total_chunks)

    idx_in = np.zeros((N_CORES, 128, total_chunks * 8), np.int16)
    msk_in = np.zeros((N_CORES, 128, total_chunks * 4 * NB), ml_dtypes.bfloat16)
    roi_of = np.zeros((N_CORES, RPC), np.int64)  # (core,row) -> original roi

    for core in range(N_CORES):
        flat_pair = np.zeros(total_chunks * 128, np.int64)
        for s in range(GPC):
            a, b = pairs[grank[s * N_CORES + core]]
            roi_of[core, 2 * s] = a
            roi_of[core, 2 * s + 1] = b
            cbase = sum(schedule[:s])
            nslot = schedule[s] * 128
            pslot = np.zeros(nslot, np.int64)
            na, nbp = len(upair[a]), len(upair[b])
            pslot[:na] = upair[a]
            pslot[na : na + nbp] = upair[b]
            flat_pair[cbase * 128 : cbase * 128 + nslot] = pslot
            # mask block [nslot, 2(sub), 2*NB]
            mblk = np.zeros((nslot, 2, 2 * NB), np.float32)
            mblk[:na, :, :NB] = umask[a]
            mblk[na : na + nbp, :, NB:] = umask[b]
            for j in range(schedule[s]):
                c = cbase + j
                blk = mblk[j * 128 : (j + 1) * 128]  # [128, 2, 98]
                for sub in range(2):
                    msk_in[core, :, (c * 2 + sub) * 2 * NB
                           : (c * 2 + sub + 1) * 2 * NB] = \
                        blk[:, sub, :].astype(ml_dtypes.bfloat16)
        # 16-lane wrapped indices per gather split, replicated across the
        # 8 lane groups
        cbase = 0
        for nch in splits:
            loc = flat_pair[cbase * 128 : (cbase + nch) * 128]
            cols = loc.reshape(-1, 16)  # desc i = m*16 + l
            for grp in range(8):
                idx_in[core, 16 * grp : 16 * grp + 16,
                       cbase * 8 : cbase * 8 + cols.shape[0]] = \
                    cols.T.astype(np.int16)
            cbase += nch

    return schedule, idx_in, msk_in, roi_of


_cache = {}


def _program(schedule, reps):
    key = (schedule, reps)
    if key not in _cache:
        _cache[key] = build_program(schedule, reps)
    return _cache[key]


_pre_cache = {}


def _prepared(data, rois, offset):
    key = (rois.tobytes(), offset.tobytes())
    if key not in _pre_cache:
        _pre_cache.clear()
        schedule, idx_in, msk_in, roi_of = _precompute(rois, offset)
        data_t = np.ascontiguousarray(
            data.transpose(0, 2, 3, 1)
        ).reshape(NPX // 2, 2 * CH).astype(ml_dtypes.bfloat16)
        _pre_cache[key] = (schedule, idx_in, msk_in, roi_of, data_t)
    return _pre_cache[key]


def run(data, rois, offset, reps: int = 1, **spmd_kwargs):
    data = np.asarray(data, dtype=np.float32)
    rois = np.asarray(rois, dtype=np.float32)
    offset = np.asarray(offset, dtype=np.float32)
    n_rois = rois.shape[0]
    assert n_rois == N_ROIS and data.shape == (B, CH, H, W)

    schedule, idx_in, msk_in, roi_of, data_t = _prepared(data, rois, offset)

    in_maps = []
    for c in range(N_CORES):
        in_maps.append({
            "data_t": data_t,
            "idxg": idx_in[c],
            "masks": msk_in[c],
        })
    res = run_bass_kernel_spmd(
        _program(schedule, reps), in_maps,
        core_ids=list(range(N_CORES)), **spmd_kwargs
    )
    out = np.zeros((n_rois, NB, CH), np.float32)
    for c in range(N_CORES):
        o = np.asarray(res.results[c]["out"]).astype(np.float32)
        o = o.reshape(2, NB, GPC, CH)  # [roi-in-group, bin, group, ch]
        for g in range(GPC):
            out[roi_of[c, 2 * g]] = o[0, :, g, :]
            out[roi_of[c, 2 * g + 1]] = o[1, :, g, :]
    out = out.transpose(0, 2, 1).reshape(n_rois, CH, P, P)
    return np.ascontiguousarray(out), res


def kernel(data, rois, offset):
    out, _ = run(data, rois, offset)
    return out


# revision 19
# speedup vs baseline: 1.2181x; 1.2181x over previous
"""Deformable PS-ROI pooling on Trainium2 (Bass/Tile), SPMD over 8 cores.

v3 strategy (host-precomputed dedup gather, pixel-pair records):
  The 784 bilinear corner reads per roi (49 bins x 2x2 samples x 4 corners)
  hit only ~105 unique feature-map pixels on average, and those cluster into
  ~55 aligned pixel PAIRS (rows are contiguous in the channel-last layout).
  The host dedups terms into per-roi unique aligned pairs and a folded
  weight matrix (bilinear x validity x 1/count), pairs rois (large+small)
  into 2-roi groups, and deals groups across cores so all 8 cores run one
  identical SPMD program.  The device executes: int16-indexed dma_gather of
  bf16 pair records (1 KiB/descriptor, ~35 chunks of 128 pairs) -> per group
  PE matmuls mask[128,98]^T x pixels[128,256] (two per chunk: even/odd pair
  halves) accumulated in PSUM -> DVE copy to bf16 -> batched output DMA
  (one per 4 groups, partition-major out layout).
"""

import math

import ml_dtypes
import numpy as np

import concourse.bacc as bacc
import concourse.mybir as mybir
from concourse import tile
from concourse.bass_utils import run_bass_kernel_spmd

F32 = mybir.dt.float32
BF16 = mybir.dt.bfloat16
I16 = mybir.dt.int16

N_CORES = 8
N_ROIS = 512
P = 7                   # pooled output size
NB = P * P              # 49 bins
CH = 256                # channels
H = W = 128             # feature map spatial
B = 2                   # batch
NPX = B * H * W         # 32768 flat pixels; 16384 aligned pairs
S = 2                   # samples per part (per axis)
RPC = N_ROIS // N_CORES         # 64 rois per core
GPC = RPC // 2                  # 32 groups (of 2 rois) per core
OBATCH = 4                      # groups per output DMA
CPG_MAX = 8                     # chunks per gather instruction (1024 descs)
SCALE = 0.0625
TRANS_STD = 0.1


def _gather_splits(total_chunks):
    """Split the chunk stream into gather instructions (<=8 chunks each),
    sized so the round-robin queue assignment balances chunk load."""
    nsplit = math.ceil(total_chunks / CPG_MAX)
    nsplit = math.ceil(nsplit / 4) * 4  # multiple of 4 for queue balance
    base = total_chunks // nsplit
    rem = total_chunks - base * nsplit
    return [base + (1 if k < rem else 0) for k in range(nsplit)]


def build_program(schedule, reps: int = 1, mode: str = "full"):
    """schedule: tuple of chunks-per-group (len GPC), identical on all cores.
    One chunk = 128 aligned pixel pairs.  mode: 'full' | 'gather' | 'compute'
    ('gather'/'compute' are timing-only variants used by the bench)."""
    total_chunks = sum(schedule)
    splits = _gather_splits(total_chunks)
    ngather = len(splits)

    nc = bacc.Bacc("TRN2", target_bir_lowering=False, debug=False, num_swdge_queues=4)
    nc.dynamic_dma_scratch_size = 2 ** 16

    data = nc.dram_tensor("data_t", [NPX // 2, 2 * CH], BF16, kind="ExternalInput")
    idx_d = nc.dram_tensor("idxg", [128, total_chunks * 8], I16,
                           kind="ExternalInput")
    msk_d = nc.dram_tensor("masks", [128, total_chunks * 4 * NB], BF16,
                           kind="ExternalInput")
    # partition-major output: out[p, g*CH + c] for bin-row p of group g
    out_d = nc.dram_tensor("out", [2 * NB, GPC * CH], BF16, kind="ExternalOutput")

    with tile.TileContext(nc) as tc:
        with (
            tc.tile_pool(name="const", bufs=1) as cst,
            tc.tile_pool(name="gp", bufs=ngather) as gp,
            tc.tile_pool(name="obp", bufs=3) as obp,
            tc.tile_pool(name="psp", bufs=6, space="PSUM") as psp,
        ):
            idxg = cst.tile([128, total_chunks * 8], I16)
            nc.sync.dma_start(idxg[:, :], idx_d.ap())
            msk = cst.tile([128, total_chunks * 4 * NB], BF16)
            nc.sync.dma_start(msk[:, :], msk_d.ap())

            from contextlib import nullcontext
            if mode == "compute":
                gt0 = cst.tile([128, max(splits) * 2 * CH], BF16)
                nc.vector.memset(gt0[:, :], 0.0)

            split_base = [0]
            for s_ in splits:
                split_base.append(split_base[-1] + s_)

            loop_cm = tc.For_i(0, reps, 1) if reps > 1 else nullcontext()
            with loop_cm:
                gts = []
                if mode in ("full", "gather"):
                    for k in range(ngather):
                        nch = splits[k]
                        gt = gp.tile([128, nch * 2 * CH], BF16)
                        dest = gt[:, :].rearrange("p (j f) -> p j f", f=2 * CH)
                        nc.gpsimd.dma_gather(
                            dest,
                            data.ap(),
                            idxg[:, split_base[k] * 8 : split_base[k + 1] * 8],
                            nch * 128,
                            nch * 128,
                            2 * CH,
                            queue_num=k % 4,
                            single_packet=False,
                        )
                        gts.append(gt)
                else:
                    gts = [gt0] * ngather

                def chunk_view(c):
                    k = 0
                    while split_base[k + 1] <= c:
                        k += 1
                    return gts[k], c - split_base[k]

                chunk = 0
                for t in range(GPC // OBATCH) if mode != "gather" else []:
                    sb = obp.tile([2 * NB, OBATCH * CH], BF16)
                    for i in range(OBATCH):
                        g = t * OBATCH + i
                        ncg = schedule[g]
                        ps = psp.tile([2 * NB, CH], F32)
                        nmm = 2 * ncg
                        for j in range(ncg):
                            c = chunk + j
                            gt, sl = chunk_view(c)
                            for s in range(2):
                                gv = gt[:, sl * 2 * CH + s * CH
                                        : sl * 2 * CH + (s + 1) * CH]
                                mm = 2 * j + s
                                nc.tensor.matmul(
                                    ps[:, :],
                                    msk[:, (c * 2 + s) * 2 * NB
                                        : (c * 2 + s + 1) * 2 * NB],
                                    gv,
                                    start=(mm == 0),
                                    stop=(mm == nmm - 1),
                                )
                        chunk += ncg
                        nc.vector.tensor_copy(
                            sb[:, i * CH : (i + 1) * CH], ps[:, :]
                        )
                    nc.sync.dma_start(
                        out_d.ap()[:, t * OBATCH * CH : (t + 1) * OBATCH * CH],
                        sb[:, :],
                    )

    nc.finalize()
    return nc


def _precompute(rois, offset):
    """Host-side: per-roi unique aligned pixel pairs + folded weight masks,
    paired and dealt into a uniform per-core schedule."""
    N = rois.shape[0]
    bidx = rois[:, 0].astype(np.int64)
    sw = np.round(rois[:, 1]) * SCALE - 0.5
    sh = np.round(rois[:, 2]) * SCALE - 0.5
    ew = (np.round(rois[:, 3]) + 1.0) * SCALE - 0.5
    eh = (np.round(rois[:, 4]) + 1.0) * SCALE - 0.5
    rw = np.maximum(ew - sw, 0.1)
    rh = np.maximum(eh - sh, 0.1)
    bw, bh = rw / P, rh / P
    subw, subh = bw / S, bh / S
    pw = np.arange(P)
    ph = np.arange(P)
    tx = offset[:, 0] * TRANS_STD
    ty = offset[:, 1] * TRANS_STD
    ws = pw[None, None, :] * bw[:, None, None] + sw[:, None, None] \
        + tx * rw[:, None, None]
    hs = ph[None, :, None] * bh[:, None, None] + sh[:, None, None] \
        + ty * rh[:, None, None]
    w = ws[..., None, None] + np.arange(S)[None, None, None, None, :] \
        * subw[:, None, None, None, None]
    h = hs[..., None, None] + np.arange(S)[None, None, None, :, None] \
        * subh[:, None, None, None, None]
    valid = (w >= -0.5) & (w <= W - 0.5) & (h >= -0.5) & (h <= H - 0.5)
    wc = np.clip(w, 0, W - 1)
    hc = np.clip(h, 0, H - 1)
    x1 = np.floor(wc).astype(np.int64)
    y1 = np.floor(hc).astype(np.int64)
    x2 = np.minimum(x1 + 1, W - 1)
    y2 = np.minimum(y1 + 1, H - 1)
    dx = (wc - x1).astype(np.float32)
    dy = (hc - y1).astype(np.float32)
    count = valid.sum(axis=(-1, -2)).astype(np.float32)
    inv = np.where(count > 0, 1.0 / np.maximum(count, 1.0), 0.0)
    v = valid.astype(np.float32) * inv[:, :, :, None, None]
    w11 = (1 - dx) * (1 - dy) * v
    w12 = (1 - dx) * dy * v
    w21 = dx * (1 - dy) * v
    w22 = dx * dy * v
    base = (bidx * (H * W))[:, None, None, None, None]

    def flat(y, x):
        return base + y * W + x

    pix = np.stack([flat(y1, x1), flat(y2, x1), flat(y1, x2), flat(y2, x2)],
                   axis=-1).reshape(N, NB, 16)
    wt = np.stack([w11, w12, w21, w22], axis=-1).reshape(N, NB, 16)

    # dedup at aligned-pair granularity; mask rows keyed (pair, sub)
    upair, umask, ucnt = [], [], np.zeros(N, np.int64)
    binc = np.repeat(np.arange(NB), 16)
    for n in range(N):
        pairs_n = pix[n] // 2
        subs_n = pix[n] & 1
        u, inv_idx = np.unique(pairs_n, return_inverse=True)
        m = np.zeros((len(u), 2, NB), np.float32)
        np.add.at(m, (inv_idx.reshape(-1), subs_n.reshape(-1), binc),
                  wt[n].reshape(-1))
        upair.append(u)
        umask.append(m)
        ucnt[n] = len(u)

    # pair rois -> groups, capacity-aware: largest partner that keeps the
    # pair-count sum within one 128-pair chunk; else largest fitting two
    # chunks (minimizes padded chunk slots)
    avail = [int(x) for x in np.argsort(-ucnt, kind="stable")]
    pairs = []
    while avail:
        a = avail.pop(0)
        b = None
        for cap in (128, 256, 1 << 30):
            for i, x in enumerate(avail):
                if ucnt[a] + ucnt[x] <= cap:
                    b = avail.pop(i)
                    break
            if b is not None:
                break
        pairs.append((a, b))
    gcost = np.array([int(math.ceil((ucnt[a] + ucnt[b]) / 128)) for a, b in pairs])
    grank = np.argsort(-gcost, kind="stable")
    schedule = tuple(
        int(gcost[grank[s * N_CORES : (s + 1) * N_CORES]].max())
        for s in range(GPC)
    )
    total_chunks = sum(schedule)
    splits = _gather_splits(total_chunks)

    idx_in = np.zeros((N_CORES, 128, total_chunks * 8), np.int16)
    msk_in = np.zeros((N_CORES, 128, total_chunks * 4 * NB), ml_dtypes.bfloat16)
    roi_of = np.zeros((N_CORES, RPC), np.int64)  # (core,row) -> original roi

    for core in range(N_CORES):
        flat_pair = np.zeros(total_chunks * 128, np.int64)
        for s in range(GPC):
            a, b = pairs[grank[s * N_CORES + core]]
            roi_of[core, 2 * s] = a
            roi_of[core, 2 * s + 1] = b
            cbase = sum(schedule[:s])
            nslot = schedule[s] * 128
            pslot = np.zeros(nslot, np.int64)
            na, nbp = len(upair[a]), len(upair[b])
            pslot[:na] = upair[a]
            pslot[na : na + nbp] = upair[b]
            flat_pair[cbase * 128 : cbase * 128 + nslot] = pslot
            # mask block [nslot, 2(sub), 2*NB]
            mblk = np.zeros((nslot, 2, 2 * NB), np.float32)
            mblk[:na, :, :NB] = umask[a]
            mblk[na : na + nbp, :, NB:] = umask[b]
            for j in range(schedule[s]):
                c = cbase + j
                blk = mblk[j * 128 : (j + 1) * 128]  # [128, 2, 98]
                for sub in range(2):
                    msk_in[core, :, (c * 2 + sub) * 2 * NB
                           : (c * 2 + sub + 1) * 2 * NB] = \
                        blk[:, sub, :].astype(ml_dtypes.bfloat16)
        # 16-lane wrapped indices per gather split, replicated across the
        # 8 lane groups
        cbase = 0
        for nch in splits:
            loc = flat_pair[cbase * 128 : (cbase + nch) * 128]
            cols = loc.reshape(-1, 16)  # desc i = m*16 + l
            for grp in range(8):
                idx_in[core, 16 * grp : 16 * grp + 16,
                       cbase * 8 : cbase * 8 + cols.shape[0]] = \
                    cols.T.astype(np.int16)
            cbase += nch

    return schedule, idx_in, msk_in, roi_of


_cache = {}


def _program(schedule, reps):
    key = (schedule, reps)
    if key not in _cache:
        _cache[key] = build_program(schedule, reps)
    return _cache[key]


_pre_cache = {}


def _prepared(data, rois, offset):
    key = (rois.tobytes(), offset.tobytes())
    if key not in _pre_cache:
        _pre_cache.clear()
        schedule, idx_in, msk_in, roi_of = _precompute(rois, offset)
        data_t = np.ascontiguousarray(
            data.transpose(0, 2, 3, 1)
        ).reshape(NPX // 2, 2 * CH).astype(ml_dtypes.bfloat16)
        _pre_cache[key] = (schedule, idx_in, msk_in, roi_of, data_t)
    return _pre_cache[key]


def run(data, rois, offset, reps: int = 1, **spmd_kwargs):
    data = np.asarray(data, dtype=np.float32)
    rois = np.asarray(rois, dtype=np.float32)
    offset = np.asarray(offset, dtype=np.float32)
    n_rois = rois.shape[0]
    assert n_rois == N_ROIS and data.shape == (B, CH, H, W)

    schedule, idx_in, msk_in, roi_of, data_t = _prepared(data, rois, offset)

    in_maps = []
    for c in range(N_CORES):
        in_maps.append({
            "data_t": data_t,
            "idxg": idx_in[c],
            "masks": msk_in[c],
        })
    res = run_bass_kernel_spmd(
        _program(schedule, reps), in_maps,
        core_ids=list(range(N_CORES)), **spmd_kwargs
    )
    out = np.zeros((n_rois, NB, CH), np.float32)
    for c in range(N_CORES):
        o = np.asarray(res.results[c]["out"]).astype(np.float32)
        o = o.reshape(2, NB, GPC, CH)  # [roi-in-group, bin, group, ch]
        for g in range(GPC):
            out[roi_of[c, 2 * g]] = o[0, :, g, :]
            out[roi_of[c, 2 * g + 1]] = o[1, :, g, :]
    out = out.transpose(0, 2, 1).reshape(n_rois, CH, P, P)
    return np.ascontiguousarray(out), res


def kernel(data, rois, offset):
    out, _ = run(data, rois, offset)
    return out
